# revision 62
# baseline (speedup 1.0000x reference)
"""AttentionWithRoPE on 8 trn2 NeuronCores.

Sharding (tensor-parallel over heads x data-parallel over batch):
  core c -> batch b = c // 4, head group g = c % 4 (heads [4g, 4g+4)).
Each core computes q/k/v projections for its 4 heads (columns
[512g, 512g+512) of Wq/Wk/Wv), causal attention with RoPE, and the
partial o_proj contribution attn_out_local @ Wo[512g:512g+512, :].
The host gather sums the 4 partials per batch (row-parallel linear).

Fused single pass, all matmul operands in bf16 (fp32 PSUM accumulate,
measured rel-err 3.4e-3, gate is 2e-2):
  - all weights + kT + v resident in SBUF (bf16 halves the footprint),
    hs streamed once per 512-query block, qT never spilled to DRAM.
  - per j block: k/q projections (RoPE fused into the PSUM eviction,
    rotate-half via sbuf->sbuf DMA on the scalar HWDGE queue), v
    projection, causal attention, o_proj rows.
  - softmax row-sums accumulate on DVE (tensor_add over exp tiles)
    instead of 160 PE ones-matmuls; one lsum matmul per head.
  - causal band blocks compute only the live column range (partial
    score/exp/AV widths) with a single 128x128 triangular mask.
  - attention is exp(ACT)-throughput-limited, so o_proj of block j-1
    is deferred and interleaved into attention(j)'s score stream; each
    head's norm chain (lsum -> 1/l -> broadcast) likewise defers into
    the next head's stream. The PE never waits on ACT/DVE.
  - single PSUM pool, 8 banks: pj(2, shared by proj/o_proj/l/bc) +
    sc(4) + av(2); sc ring lets the PE run 4 score blocks ahead.
  - simulated 323 us vs 492 us for the fp32r two-phase baseline.
"""

import sys

for _p in ("/opt/trn_rl_repo", "/root/.axon_site/_ro/trn_rl_repo"):
    if _p not in sys.path:
        sys.path.insert(0, _p)

import numpy as np
import ml_dtypes

import concourse.bass as bass
import concourse.tile as tile
from concourse import bacc, mybir
from concourse.bass_utils import run_bass_kernel_spmd

f32 = mybir.dt.float32
f32r = mybir.dt.float32r
bf16 = mybir.dt.bfloat16
np_bf16 = ml_dtypes.bfloat16
EXP = mybir.ActivationFunctionType.Exp
COPY = mybir.ActivationFunctionType.Copy

B = 2
S = 2048
E = 2048
D = 128
HL = 4          # local heads per core
EL = HL * D     # 512, local projection width
NB = S // 512   # 4 query/key 512-blocks
EC = E // 128   # 16 contraction chunks
SCALE = float(1.0 / np.sqrt(D))

_CACHE = {}
TRACE = False
LAST_EXEC_NS = None
LAST_RESULT = None


def _build():
    from contextlib import ExitStack

    nc = bacc.Bacc("TRN2", target_bir_lowering=False, debug=False, num_devices=8)

    HS = nc.dram_tensor("hs", [NB, 128, EC, 512], bf16, kind="ExternalInput")
    # wq/wk are per-head-major so head 0's projection unblocks after 0.5MB
    WQ = nc.dram_tensor("wq", [HL, 128, EC, 128], bf16, kind="ExternalInput")
    WK = nc.dram_tensor("wk", [HL, 128, EC, 128], bf16, kind="ExternalInput")
    WV = nc.dram_tensor("wv", [128, EC, EL], bf16, kind="ExternalInput")
    WO = nc.dram_tensor("wo", [128, HL, E], bf16, kind="ExternalInput")
    COS = nc.dram_tensor("cosb", [NB, 128, 512], bf16, kind="ExternalInput")
    SIN = nc.dram_tensor("sinb", [NB, 128, 512], bf16, kind="ExternalInput")  # sign-folded
    MSK = nc.dram_tensor("tri", [128, 128], bf16, kind="ExternalInput")
    ONC = nc.dram_tensor("onec", [128, 1], bf16, kind="ExternalInput")
    ONR = nc.dram_tensor("oner", [1, 128], f32r, kind="ExternalInput")
    OUT = nc.dram_tensor("out", [S, E], f32, kind="ExternalOutput")

    with tile.TileContext(nc) as tc, nc.allow_low_precision("bf16 compute by design"):
        with ExitStack() as octx:
            res = octx.enter_context(tc.tile_pool(name="res", bufs=1))
            wq_sb = res.tile([128, HL, EC, 128], bf16, tag="wq")
            wk_sb = res.tile([128, HL, EC, 128], bf16, tag="wk")
            wv_sb = res.tile([128, EC, EL], bf16, tag="wv")
            wo_sb = res.tile([128, HL, E], bf16, tag="wo")
            kT = [res.tile([128, S], bf16, tag=f"kT{h}", name=f"kT{h}") for h in range(HL)]
            v_sb = res.tile([128, NB * 4, EL], bf16, tag="v")
            tri = res.tile([128, 128], bf16, tag="tri")
            onec = res.tile([128, 1], bf16, tag="onec")
            oner = res.tile([1, 128], f32r, tag="oner")

            hsp = octx.enter_context(tc.tile_pool(name="hsp", bufs=2))
            csp = octx.enter_context(tc.tile_pool(name="csp", bufs=2))
            rtp = octx.enter_context(tc.tile_pool(name="rtp", bufs=2))
            qtp = octx.enter_context(tc.tile_pool(name="qtp", bufs=4))
            exq = octx.enter_context(tc.tile_pool(name="exq", bufs=3))
            onp = octx.enter_context(tc.tile_pool(name="onp", bufs=2))
            orp = octx.enter_context(tc.tile_pool(name="orp", bufs=3))
            bcp = octx.enter_context(tc.tile_pool(name="bcp", bufs=2))
            ps = octx.enter_context(tc.tile_pool(name="ps", bufs=2, space="PSUM"))

            def load_hs(j, chunks=1):
                t = hsp.tile([128, EC, 512], bf16, tag="hs", name=f"hs{j}")
                step = EC // chunks
                for cc in range(chunks):
                    nc.sync.dma_start(
                        t[:, cc * step:(cc + 1) * step, :],
                        HS[j, :, cc * step:(cc + 1) * step, :],
                    )
                return t

            def load_cs(j):
                c = csp.tile([128, 512], bf16, tag="cos", name=f"cos{j}")
                s = csp.tile([128, 512], bf16, tag="sin", name=f"sin{j}")
                nc.sync.dma_start(c[:], COS[j])
                nc.sync.dma_start(s[:], SIN[j])
                return c, s

            # prologue, ordered by first PE use: wk head0 -> hs0 (chunked) ->
            # cos/sin -> wk heads 1-3 -> wq per head -> wv. wo + hs1 are
            # emitted later (mid-attention of j=0) so they don't delay the
            # rope rot DMAs.
            nc.sync.dma_start(wk_sb[:, 0], WK[0])
            hs_cur = load_hs(0, chunks=4)
            cs_cur = load_cs(0)
            for hh in range(1, HL):
                nc.sync.dma_start(wk_sb[:, hh], WK[hh])
            for hh in range(HL):
                nc.sync.dma_start(wq_sb[:, hh], WQ[hh])
            for cc in range(4):
                nc.sync.dma_start(
                    wv_sb[:, 4 * cc:4 * cc + 4, :], WV[:, 4 * cc:4 * cc + 4, :]
                )
            nc.scalar.dma_start(tri[:], MSK[:])
            nc.scalar.dma_start(onec[:], ONC[:])
            nc.scalar.dma_start(oner[:], ONR[:])

            def rope_evict(dst, ps_t, cos_t, sin_t, nm):
                # dst = raw*cosT + rot(raw)*sinT_signed (signs folded on host)
                raw = rtp.tile([128, 512], bf16, tag="qkraw", name=f"raw{nm}")
                nc.scalar.activation(raw[:], ps_t[:], COPY)
                rot = rtp.tile([128, 512], bf16, tag="qkrot", name=f"rot{nm}")
                # rotate-half on the scalar HWDGE queue: keeps these small
                # latency-critical moves off the bulk-load (sync) queue
                nc.scalar.dma_start(rot[0:64, :], raw[64:128, :])
                nc.scalar.dma_start(rot[64:128, :], raw[0:64, :])
                t1 = rtp.tile([128, 512], bf16, tag="ropet1", name=f"t1{nm}")
                nc.vector.tensor_mul(t1[:], raw[:], cos_t[:])
                nc.vector.tensor_mul(dst, rot[:], sin_t[:])
                nc.vector.tensor_add(dst, dst, t1[:])

            def make_op_emitters(jj, onorm_list):
                # o_proj partial rows for block jj, as deferred emitters that
                # interleave into the next block's attention: the PE fills
                # its exp-wait gaps with o_proj matmuls instead of idling
                thunks = []
                for i in range(4):
                    for n in range(4):
                        def em(i=i, n=n, jj=jj, onorm_list=onorm_list):
                            op = ps.tile(
                                [128, 512], f32, tag="pj", bufs=2,
                                name=f"op{jj}_{i}_{n}",
                            )
                            for hh in range(HL):
                                nc.tensor.matmul(
                                    op[:],
                                    onorm_list[hh][:, i * 128:(i + 1) * 128],
                                    wo_sb[:, hh, n * 512:(n + 1) * 512],
                                    start=(hh == 0),
                                    stop=(hh == HL - 1),
                                )
                            orow = orp.tile(
                                [128, 512], f32, tag="orow", bufs=6,
                                name=f"or{jj}_{i}_{n}",
                            )
                            if n % 2 == 0:
                                nc.vector.tensor_copy(orow[:], op[:])
                                eng = nc.sync
                            else:
                                nc.scalar.activation(orow[:], op[:], COPY)
                                eng = nc.scalar
                            eng.dma_start(
                                OUT[jj * 512 + i * 128:jj * 512 + (i + 1) * 128,
                                    n * 512:(n + 1) * 512],
                                orow[:],
                            )
                        thunks.append(em)
                return thunks

            oproj_thunks = []
            pending = None  # deferred norm chain (av, exacc -> l -> 1/l)

            def emit_lsum_fn(p):
                l_t = ps.tile([1, 512], f32, tag="pj", bufs=2, name=f"l{p['tag']}")
                nc.tensor.matmul(l_t[:], onec[:], p["exacc"][:], start=True, stop=True)
                p["l"] = l_t

            def flush_norm_fn(p):
                av_t, l_t, tg = p["av"], p["l"], p["tag"]
                recip = bcp.tile([1, 512], f32r, tag="recip", name=f"rc{tg}")
                nc.vector.reciprocal(recip[:], l_t[:])
                bc = ps.tile([128, 512], f32, tag="pj", bufs=2, name=f"bc{tg}")
                nc.tensor.matmul(bc[:], oner[:], recip[:], start=True, stop=True)
                bc_sb = bcp.tile([128, 512], f32, tag="bcsb", name=f"bcs{tg}")
                nc.vector.tensor_copy(bc_sb[:], bc[:])
                on = onp.tile([128, 512], bf16, tag=f"on{p['h']}", name=f"on{tg}")
                nc.vector.tensor_mul(on[:], av_t[:], bc_sb[:])
                p["onorm_list"].append(on)

            for j in range(NB):
                jc = slice(j * 512, (j + 1) * 512)

                # ---- projections for block j (k, q, v; the previous
                # block's h3 norm chain flushes under k-proj cover) ----
                for h in range(HL):
                    kps = ps.tile([128, 512], f32, tag="pj", bufs=2, name=f"kps{j}_{h}")
                    for e in range(EC):
                        nc.tensor.matmul(
                            kps[:],
                            wk_sb[:, h, e, :],
                            hs_cur[:, e, :],
                            start=(e == 0),
                            stop=(e == EC - 1),
                        )
                    rope_evict(kT[h][:, jc], kps, cs_cur[0], cs_cur[1], f"k{j}_{h}")
                    if h == 0 and pending is not None:
                        emit_lsum_fn(pending)
                    if h == 1 and pending is not None:
                        flush_norm_fn(pending)
                        oproj_thunks = make_op_emitters(
                            pending["jj"], pending["onorm_list"]
                        )
                        pending = None

                qt = []
                for h in range(HL):
                    qps = ps.tile([128, 512], f32, tag="pj", bufs=2, name=f"qps{j}_{h}")
                    for e in range(EC):
                        nc.tensor.matmul(
                            qps[:],
                            wq_sb[:, h, e, :],
                            hs_cur[:, e, :],
                            start=(e == 0),
                            stop=(e == EC - 1),
                        )
                    qh = qtp.tile([128, 512], bf16, tag="qt", name=f"qt{j}_{h}")
                    rope_evict(qh[:], qps, cs_cur[0], cs_cur[1], f"q{j}_{h}")
                    qt.append(qh)

                for i in range(4):
                    vps = ps.tile([128, 512], f32, tag="pj", bufs=2, name=f"vps{j}_{i}")
                    for e in range(EC):
                        nc.tensor.matmul(
                            vps[:],
                            hs_cur[:, e, i * 128:(i + 1) * 128],
                            wv_sb[:, e, :],
                            start=(e == 0),
                            stop=(e == EC - 1),
                        )
                    nc.vector.tensor_copy(v_sb[:, 4 * j + i, :], vps[:])

                # ---- attention for block j ----
                nkb = 4 * (j + 1)
                onorm = []
                step = max(1, (nkb - 4) // 4)
                pop_points = {1, 3, 3 + step, 3 + 2 * step}

                for h in range(HL):
                    av = ps.tile([128, 512], f32, tag="av", bufs=2, name=f"av{j}_{h}")
                    exacc = exq.tile(
                        [128, 512], bf16, tag="exacc", bufs=2, name=f"exa{j}_{h}"
                    )
                    scq = []

                    def emit_av(item):
                        kb, ex, lo = item
                        nc.tensor.matmul(
                            av[:, lo:512],
                            v_sb[:, kb, h * 128:(h + 1) * 128],
                            ex[:, lo:512],
                            start=(kb == 0),
                            stop=(kb == nkb - 1),
                            skip_group_check=True,
                        )

                    for kb in range(nkb):
                        m = kb - 4 * j
                        lo = 128 * m if m > 0 else 0  # band: cols < lo are masked
                        sc = ps.tile(
                            [128, 512], f32, tag="sc", bufs=4, name=f"sc{j}_{h}_{kb}"
                        )
                        nc.tensor.matmul(
                            sc[:, lo:512],
                            kT[h][:, kb * 128:(kb + 1) * 128],
                            qt[h][:, lo:512],
                            start=True,
                            stop=True,
                        )
                        ex = exq.tile(
                            [128, 512], bf16, tag="ex", bufs=6, name=f"ex{j}_{h}_{kb}"
                        )
                        nc.scalar.activation(ex[:, lo:512], sc[:, lo:512], EXP, scale=SCALE)
                        if m >= 0:  # diagonal 128x128 square: triangular mask
                            nc.gpsimd.tensor_mul(
                                ex[:, lo:lo + 128], ex[:, lo:lo + 128], tri[:]
                            )
                        # row-sum accumulation on DVE (frees 1 PE pass per kb)
                        if kb == 0:
                            nc.gpsimd.tensor_copy(exacc[:], ex[:])
                        else:
                            nc.vector.tensor_add(
                                exacc[:, lo:512], exacc[:, lo:512], ex[:, lo:512]
                            )
                        scq.append((kb, ex, lo))
                        # previous head's norm chain interleaves into this
                        # head's score stream so the PE never waits on the
                        # DVE row-sum accumulation or the reciprocal
                        if kb == 2 and pending is not None:
                            emit_lsum_fn(pending)
                        if kb == 3 and pending is not None:
                            flush_norm_fn(pending)
                            pending = None
                        if len(scq) >= 4:
                            emit_av(scq.pop(0))
                        if kb in pop_points and oproj_thunks:
                            oproj_thunks.pop(0)()
                    while scq:
                        emit_av(scq.pop(0))
                    pending = {
                        "av": av, "exacc": exacc, "h": h,
                        "tag": f"{j}_{h}", "jj": j, "onorm_list": onorm,
                    }
                    if h == 0:
                        # queue bulk loads for the next phases now: the rope
                        # rot DMAs of this j are already in flight, and these
                        # arrive well before o_proj / the next projections
                        if j == 0:
                            nc.sync.dma_start(wo_sb[:], WO[:])
                        if j + 1 < NB:
                            hs_nxt = load_hs(j + 1)
                            cs_nxt = load_cs(j + 1)
                assert not oproj_thunks  # all of j-1's o_proj emitted

                if j + 1 < NB:
                    hs_cur = hs_nxt
                    cs_cur = cs_nxt

            # final block: h3 norm chain + o_proj(3), nothing left to hide in
            emit_lsum_fn(pending)
            flush_norm_fn(pending)
            for t in make_op_emitters(pending["jj"], pending["onorm_list"]):
                t()
            pending = None

    nc.compile()
    return nc


def _get_nc():
    if "nc" not in _CACHE:
        _CACHE["nc"] = _build()
    return _CACHE["nc"]


def _make_tri():
    sk = np.arange(128)[:, None]
    sq = np.arange(128)[None, :]
    return (sq >= sk).astype(np_bf16)


def _prep_in_maps(hidden_states, cos, sin, Wq, Wk, Wv, Wo):
    hidden_states = np.asarray(hidden_states, dtype=np.float32)
    cos = np.asarray(cos, dtype=np.float32)
    sin = np.asarray(sin, dtype=np.float32)
    Wq = np.asarray(Wq, dtype=np.float32)
    Wk = np.asarray(Wk, dtype=np.float32)
    Wv = np.asarray(Wv, dtype=np.float32)
    Wo = np.asarray(Wo, dtype=np.float32)

    tri = _make_tri()
    onec = np.ones((128, 1), dtype=np_bf16)
    oner = np.ones((1, 128), dtype=np.float32)

    hs_b, cos_b, sin_b = [], [], []
    for b in range(B):
        hsT = hidden_states[b].T  # [E, S]
        hs_b.append(
            np.ascontiguousarray(
                hsT.reshape(EC, 128, NB, 512).transpose(2, 1, 0, 3)
            ).astype(np_bf16)
        )
        cT = cos[b].T  # [D, S]
        cos_b.append(
            np.ascontiguousarray(cT.reshape(128, NB, 512).transpose(1, 0, 2)).astype(
                np_bf16
            )
        )
        sT = sin[b].T.copy()
        sT[:64] *= -1.0
        sin_b.append(
            np.ascontiguousarray(sT.reshape(128, NB, 512).transpose(1, 0, 2)).astype(
                np_bf16
            )
        )

    wq_g, wk_g, wv_g, wo_g = [], [], [], []
    for g in range(4):
        cols = slice(512 * g, 512 * (g + 1))
        wq_g.append(
            np.ascontiguousarray(
                Wq[:, cols].reshape(EC, 128, HL, 128).transpose(2, 1, 0, 3)
            ).astype(np_bf16)
        )
        wk_g.append(
            np.ascontiguousarray(
                Wk[:, cols].reshape(EC, 128, HL, 128).transpose(2, 1, 0, 3)
            ).astype(np_bf16)
        )
        wv_g.append(
            np.ascontiguousarray(
                Wv[:, cols].reshape(EC, 128, EL).transpose(1, 0, 2)
            ).astype(np_bf16)
        )
        wo_g.append(
            np.ascontiguousarray(
                Wo[cols, :].reshape(HL, 128, E).transpose(1, 0, 2)
            ).astype(np_bf16)
        )

    in_maps = []
    for c in range(8):
        b, g = c // 4, c % 4
        in_maps.append({
            "hs": hs_b[b],
            "wq": wq_g[g],
            "wk": wk_g[g],
            "wv": wv_g[g],
            "wo": wo_g[g],
            "cosb": cos_b[b],
            "sinb": sin_b[b],
            "tri": tri,
            "onec": onec,
            "oner": oner,
        })
    return in_maps


def _get_exec():
    """Cached jitted SPMD executor (mirrors run_bass_via_pjrt's multi-core
    path, but reusable across calls so repeated runs skip retrace/compile)."""
    if "exec" in _CACHE:
        return _CACHE["exec"]

    import jax
    from jax.sharding import Mesh, PartitionSpec
    try:
        from jax.experimental.shard_map import shard_map
    except ImportError:
        from jax.shard_map import shard_map
    from concourse import bass2jax

    nc = _get_nc()
    bass2jax.install_neuronx_cc_hook()

    partition_name = nc.partition_id_tensor.name if nc.partition_id_tensor else None
    in_names, out_names, out_avals, zero_specs = [], [], [], []
    for alloc in nc.m.functions[0].allocations:
        if not isinstance(alloc, mybir.MemoryLocationSet):
            continue
        name = alloc.memorylocations[0].name
        if alloc.kind == "ExternalInput":
            if name != partition_name:
                in_names.append(name)
        elif alloc.kind == "ExternalOutput":
            out_names.append(name)
            shape = tuple(alloc.tensor_shape)
            dtype = mybir.dt.np(alloc.dtype)
            out_avals.append(jax.core.ShapedArray(shape, dtype))
            zero_specs.append((shape, dtype))
    n_params = len(in_names)
    n_outs = len(out_names)
    all_names = list(in_names) + list(out_names)
    if partition_name is not None:
        all_names.append(partition_name)
    donate = tuple(range(n_params, n_params + n_outs))

    def _body(*args):
        operands = list(args)
        if partition_name is not None:
            operands.append(bass2jax.partition_id_tensor())
        outs = bass2jax._bass_exec_p.bind(
            *operands,
            out_avals=tuple(out_avals),
            in_names=tuple(all_names),
            out_names=tuple(out_names),
            lowering_input_output_aliases=(),
            sim_require_finite=True,
            sim_require_nnan=True,
            nc=nc,
        )
        return tuple(outs)

    devices = jax.devices()[:8]
    mesh = Mesh(np.asarray(devices), ("core",))
    in_specs = (PartitionSpec("core"),) * (n_params + n_outs)
    out_specs = (PartitionSpec("core"),) * n_outs
    sharded = jax.jit(
        shard_map(
            _body, mesh=mesh, in_specs=in_specs, out_specs=out_specs, check_rep=False
        ),
        donate_argnums=donate,
        keep_unused=True,
    )
    _CACHE["exec"] = (sharded, in_names, out_names, out_avals, zero_specs, mesh)
    return _CACHE["exec"]


def _run_fast(in_maps):
    sharded, in_names, out_names, out_avals, zero_specs, mesh = _get_exec()
    concat_in = [
        np.concatenate([np.asarray(m[name]) for m in in_maps], axis=0)
        for name in in_names
    ]
    zeros = [np.zeros((8 * s[0], *s[1:]), d) for s, d in zero_specs]
    outs = sharded(*concat_in, *zeros)
    per_core = []
    for c in range(8):
        per_core.append({
            name: np.asarray(outs[i]).reshape(8, *out_avals[i].shape)[c]
            for i, name in enumerate(out_names)
        })
    return per_core


def _gather(results):
    out = np.empty((B, S, E), dtype=np.float32)
    for b in range(B):
        acc = results[4 * b]["out"].astype(np.float32)
        for g in range(1, 4):
            acc = acc + results[4 * b + g]["out"]
        out[b] = acc
    return out


def kernel(hidden_states, cos, sin, Wq, Wk, Wv, Wo):
    global LAST_EXEC_NS, LAST_RESULT
    in_maps = _prep_in_maps(hidden_states, cos, sin, Wq, Wk, Wv, Wo)
    try:
        results = _run_fast(in_maps)
    except Exception:
        nc = _get_nc()
        kw = {}
        if TRACE:
            import tempfile

            kw = dict(trace=True, tmpdir=tempfile.mkdtemp(prefix="bass_trace_"))
        res = run_bass_kernel_spmd(nc, in_maps, core_ids=list(range(8)), **kw)
        LAST_EXEC_NS = res.exec_time_ns
        LAST_RESULT = res
        results = res.results
    return _gather(results)


def bench(inputs, iters=30, warmup=3):
    """Repeat-execute the compiled NEFF on device-resident inputs; returns
    per-iteration wall ns (amortized, async-pipelined dispatch)."""
    import time

    import jax
    from jax.sharding import NamedSharding, PartitionSpec

    in_maps = _prep_in_maps(**inputs)
    sharded, in_names, out_names, out_avals, zero_specs, mesh = _get_exec()
    sh = NamedSharding(mesh, PartitionSpec("core"))
    dev_in = [
        jax.device_put(
            np.concatenate([np.asarray(m[name]) for m in in_maps], axis=0), sh
        )
        for name in in_names
    ]
    outs = tuple(
        jax.device_put(np.zeros((8 * s[0], *s[1:]), d), sh) for s, d in zero_specs
    )
    for _ in range(warmup):
        outs = sharded(*dev_in, *outs)
    jax.block_until_ready(outs)
    t0 = time.perf_counter()
    for _ in range(iters):
        outs = sharded(*dev_in, *outs)
    jax.block_until_ready(outs)
    t1 = time.perf_counter()
    return (t1 - t0) / iters * 1e9
